# revision 10
# baseline (speedup 1.0000x reference)
"""Trainium2 Bass kernel for DiagTrainableLDAHead (retrieval_knn).

out[n,c] = log_prior[c] - 0.5*(m2[n,c] + log_det)
m2[n,c]  = sum_d (z[n,d]-mu[c,d])^2 * inv_var[d]
=> out[n,c] = cross[n,c] + rb[n] + cb[c]
   cross = (z*inv_var) @ mu^T      (fp8e4 DoubleRow GEMM, 0.5 cyc/row)
   rb[n] = -0.5 * sum_d z^2 inv_var          (fp16 PE reduce)
   cb[c] = log_prior[c] - 0.5*(mu_sq[c]+log_det)  (fp8 DR PE reduce)

rb/cb are folded INTO the GEMM as a third DoubleRow matmul on augmented
fp8 operands (rb scaled by 1/8 and split hi/mid/lo across three k-slots
whose partner value is 8; cb split hi/lo across two k-slots) so PSUM
holds the finished output and eviction is a bare fp32->fp16 copy.

Sharding: data-parallel over N across 8 NeuronCores (1024 rows each);
mu / log_cov_diag / prior_logits replicated; forward-only, no
collectives.  Host prep is layout+dtype only (transposes, bf16/fp8
rounding); all arithmetic is on-device.

DMA: all large tensors use dense per-partition-contiguous layouts
(>=4KB descriptors).  Inputs ride the scalar HWDGE ring, outputs the
sync HWDGE ring, so loads and stores never share a FIFO.
"""
import sys

sys.path.insert(0, "/opt/trn_rl_repo")

import ml_dtypes
import numpy as np

import concourse.bacc as bacc
import concourse.tile as tile
from concourse import mybir
from concourse.bass_utils import run_bass_kernel_spmd

F32 = mybir.dt.float32
F16 = mybir.dt.float16
BF16 = mybir.dt.bfloat16
F8 = mybir.dt.float8e4
AF = mybir.ActivationFunctionType
ALU = mybir.AluOpType
DR = mybir.MatmulPerfMode.DoubleRow

N, C, D = 8192, 2048, 512
NCORES = 8
NSH = N // NCORES          # 1024 rows per core
P = 128
NCH = 2                    # n-chunks of 512
KT = 4                     # 128-wide k-tiles
KT2 = 2                    # 256-wide DoubleRow k-tiles (pairs i=0/1)
CJ = 4                     # c-chunks of 512
F = 512
NT = NSH // P              # 8 output row-tiles
WARM_PRE = 4               # PE warmup matmuls before lc transposes
WARM_POST = 4              # ... and after

_CACHE = {}


def _build():
    nc = bacc.Bacc("TRN2", target_bir_lowering=False, debug=False,
                   enable_asserts=False, num_devices=NCORES)

    # dense layouts: every DMA is >=4KB contiguous per partition
    zB = nc.dram_tensor("zB", [P, NCH, KT, F], BF16, kind="ExternalInput").ap()
    m8 = nc.dram_tensor("m8", [P, KT2, 2, C], F8, kind="ExternalInput").ap()
    lc = nc.dram_tensor("lc", [1, D], F32, kind="ExternalInput").ap()
    prior = nc.dram_tensor("prior", [1, C], F32, kind="ExternalInput").ap()
    outW = nc.dram_tensor("outW", [P, NT, C], F16, kind="ExternalOutput").ap()

    with tile.TileContext(nc) as tc:
        with (
            tc.tile_pool(name="const", bufs=1) as const,
            tc.tile_pool(name="stage", bufs=3) as stage,
            tc.tile_pool(name="psS", bufs=2, space="PSUM") as psS,
            tc.tile_pool(name="psM", bufs=6, space="PSUM") as psM,
        ):
            # ---- tiny const loads (scalar HWDGE ring) -----------------
            lc_f = const.tile([1, D], F32)
            nc.scalar.dma_start(out=lc_f[:], in_=lc[:, :])
            pr = const.tile([1, C], F32)
            nc.scalar.dma_start(out=pr[:], in_=prior[:, :])

            # ---- big input loads (scalar HWDGE ring) ------------------
            zF = const.tile([P, NCH, KT, F], BF16)
            m8s = const.tile([P, KT2, 2, C], F8)
            nc.scalar.dma_start(out=zF[:, 0], in_=zB[:, 0])
            nc.scalar.dma_start(out=m8s[:, 0], in_=m8[:, 0])
            nc.scalar.dma_start(out=zF[:, 1], in_=zB[:, 1])
            nc.scalar.dma_start(out=m8s[:, 1], in_=m8[:, 1])

            # ---- PE warmup: release the HAM throttle before real work -
            wz = const.tile([P, F], BF16)
            nc.vector.memset(wz[:], 0.0)
            for _ in range(WARM_PRE):
                pw = psS.tile([8, F], F32, tag="setup")
                nc.tensor.matmul(pw[:], lhsT=wz[:, 0:8], rhs=wz[:],
                                 start=True, stop=True)

            # lc into partition layout [p, kt] via PE transposes
            id1 = const.tile([1, 1], F32)
            nc.vector.memset(id1[:], 1.0)
            plc = psS.tile([P, KT], F32, tag="setup")
            for kt in range(KT):
                nc.tensor.transpose(plc[:, kt:kt + 1],
                                    lc_f[:, kt * P:(kt + 1) * P], id1[:])
            lc_p = const.tile([P, KT], F32)
            nc.scalar.copy(lc_p[:], plc[:])

            for _ in range(WARM_POST):
                pw = psS.tile([8, F], F32, tag="setup")
                nc.tensor.matmul(pw[:], lhsT=wz[:, 0:8], rhs=wz[:],
                                 start=True, stop=True)

            iv = const.tile([P, KT], F32)       # exp(-lc)
            nc.scalar.activation(iv[:], lc_p[:], AF.Exp, scale=-1.0)
            iv16 = const.tile([P, KT], F16)     # lhsT for z_sq reduce
            nc.scalar.copy(iv16[:], iv[:])
            # DR lhsT for mu_sq reduce; inner pad 16 so the k-pair stride
            # is 16B (DoubleRow ldweights requires step % 16 == 0)
            iv8 = const.tile([P, KT2, 2, 16], F8)
            for kt in range(KT):
                nc.scalar.copy(iv8[:, kt // 2, kt % 2, 0:1], iv[:, kt:kt + 1])

            # log_det and log_prior
            ldsum = const.tile([1, 1], F32)
            nc.vector.tensor_reduce(out=ldsum[:], in_=lc_f[:],
                                    axis=mybir.AxisListType.X, op=ALU.add)
            nldh = const.tile([1, 1], F32)      # -0.5*log_det
            nc.scalar.mul(nldh[:], ldsum[:], -0.5)

            pmax = const.tile([1, 1], F32)
            nc.vector.tensor_reduce(out=pmax[:], in_=pr[:],
                                    axis=mybir.AxisListType.X, op=ALU.max)
            npmax = const.tile([1, 1], F32)
            nc.scalar.mul(npmax[:], pmax[:], -1.0)
            pexp = const.tile([1, C], F32)
            nc.scalar.activation(pexp[:], pr[:], AF.Exp, bias=npmax[:], scale=1.0)
            psum_e = const.tile([1, 1], F32)
            nc.vector.tensor_reduce(out=psum_e[:], in_=pexp[:],
                                    axis=mybir.AxisListType.X, op=ALU.add)
            lse = const.tile([1, 1], F32)
            nc.scalar.activation(lse[:], psum_e[:], AF.Ln)
            nb = const.tile([1, 1], F32)
            nc.scalar.activation(nb[:], lse[:], AF.Identity, bias=pmax[:], scale=1.0)
            nc.scalar.mul(nb[:], nb[:], -1.0)
            lp = const.tile([1, C], F32)        # log_prior
            nc.scalar.activation(lp[:], pr[:], AF.Identity, bias=nb[:], scale=1.0)

            # ---- augmented bias operands (fp8, fed to 3rd DR matmul) --
            # slot (Ki,i): (0,0) rb_hi*8  (0,1) rb_mid*8  (32,0) rb_lo*8
            #              (32,1) 1*cb_hi  (64,0) 1*cb_lo  rest zero
            # (dynamic rows sit at partition 0/32/64 — the only legal
            #  base partitions for engine writes)
            zaug = const.tile([P, 2, NSH], F8)
            baug = const.tile([P, 2, C], F8)
            nc.vector.memset(zaug[:], 0.0)
            nc.vector.memset(baug[:], 0.0)
            nc.vector.memset(zaug[32:33, 1, :], 1.0)
            nc.vector.memset(zaug[64:65, 0, :], 1.0)
            nc.vector.memset(baug[0:1, :, :], 8.0)
            nc.vector.memset(baug[32:33, 0, :], 8.0)

            # ---- z-side prep per n-chunk ------------------------------
            zq = const.tile([P, NCH, KT, F], F16)    # z^2
            z8 = const.tile([P, NCH, KT2, 2, F], F8)  # fp8(z*iv)
            rb32 = const.tile([1, NSH], F32)
            h32 = const.tile([1, NSH], F32)
            m32 = const.tile([1, NSH], F32)
            m32b = const.tile([1, NSH], F32)
            l32 = const.tile([1, NSH], F32)

            def prep_z(ch):
                s = slice(ch * F, (ch + 1) * F)
                nc.vector.tensor_tensor(zq[:, ch], zF[:, ch], zF[:, ch],
                                        ALU.mult)
                for kt in range(KT):
                    nc.vector.tensor_scalar_mul(
                        z8[:, ch, kt // 2, kt % 2, :], zF[:, ch, kt, :],
                        iv[:, kt:kt + 1])
                # z_sq reduce on PE (fp16), then rb = -z_sq/16 split into
                # three fp8 k-slots (partner slot carries the 8x)
                pz = psS.tile([1, F], F32, tag="setup")
                for kt in range(KT):
                    nc.tensor.matmul(pz[:], lhsT=iv16[:, kt:kt + 1],
                                     rhs=zq[:, ch, kt, :],
                                     start=(kt == 0), stop=(kt == KT - 1))
                nc.scalar.activation(rb32[:, s], pz[:], AF.Copy, scale=-0.0625)
                nc.scalar.copy(zaug[0:1, 0, s], rb32[:, s])
                nc.scalar.copy(h32[:, s], zaug[0:1, 0, s])
                nc.vector.tensor_tensor(m32[:, s], rb32[:, s], h32[:, s],
                                        ALU.subtract)
                nc.scalar.copy(zaug[0:1, 1, s], m32[:, s])
                nc.scalar.copy(m32b[:, s], zaug[0:1, 1, s])
                nc.vector.tensor_tensor(l32[:, s], m32[:, s], m32b[:, s],
                                        ALU.subtract)
                nc.scalar.copy(zaug[32:33, 0, s], l32[:, s])

            # ---- mu-side prep per c-chunk -----------------------------
            mq = const.tile([P, KT2, 2, C], F8)      # mu^2 (fp8 ok: terms tiny)
            cb32 = const.tile([1, C], F32)
            ch32 = const.tile([1, C], F32)
            cl32 = const.tile([1, C], F32)

            def prep_mu(cj):
                s = slice(cj * F, (cj + 1) * F)
                # squares: kt2=0 on scalar, kt2=1 on vector (balance)
                nc.scalar.activation(mq[:, 0, :, s], m8s[:, 0, :, s], AF.Square)
                nc.vector.tensor_tensor(mq[:, 1, :, s], m8s[:, 1, :, s],
                                        m8s[:, 1, :, s], ALU.mult)
                pm = psS.tile([1, F], F32, tag="setup")
                for k2 in range(KT2):
                    nc.tensor.matmul(pm[:], lhsT=iv8[:, k2, :, 0:1],
                                     rhs=mq[:, k2, :, s],
                                     start=(k2 == 0), stop=(k2 == KT2 - 1),
                                     perf_mode=DR)
                # cb = lp - 0.5*(mu_sq + log_det), split hi/lo
                nc.scalar.activation(cb32[:, s], pm[:], AF.Identity,
                                     bias=nldh[:], scale=-0.5)
                nc.vector.tensor_tensor(cb32[:, s], cb32[:, s], lp[:, s],
                                        ALU.add)
                nc.scalar.copy(baug[32:33, 1, s], cb32[:, s])
                nc.scalar.copy(ch32[:, s], baug[32:33, 1, s])
                nc.vector.tensor_tensor(cl32[:, s], cb32[:, s], ch32[:, s],
                                        ALU.subtract)
                nc.scalar.copy(baug[64:65, 0, s], cl32[:, s])

            for ch in range(NCH):
                prep_z(ch)
            for cj in range(CJ):
                prep_mu(cj)

            # ---- main GEMM: 3 DoubleRow matmuls per (ni, cj) ----------
            def main_tile(ni):
                ch, t = ni // KT, ni % KT
                ns = slice(t * P, (t + 1) * P)
                ps = [psM.tile([P, F], F32, name=f"ps{cj}", tag="mm")
                      for cj in range(CJ)]
                for k2 in range(KT2):
                    for cj in range(CJ):
                        nc.tensor.matmul(
                            ps[cj][:],
                            lhsT=z8[:, ch, k2, :, ns],
                            rhs=m8s[:, k2, :, cj * F:(cj + 1) * F],
                            start=(k2 == 0), stop=False, perf_mode=DR)
                for cj in range(CJ):
                    nc.tensor.matmul(
                        ps[cj][:],
                        lhsT=zaug[:, :, ni * P:(ni + 1) * P],
                        rhs=baug[:, :, cj * F:(cj + 1) * F],
                        start=False, stop=True, perf_mode=DR)
                st = stage.tile([P, C], F16, tag="st")
                for cj in range(CJ):
                    if cj < 2:
                        nc.scalar.copy(st[:, cj * F:(cj + 1) * F], ps[cj][:])
                    else:
                        nc.vector.tensor_copy(out=st[:, cj * F:(cj + 1) * F],
                                              in_=ps[cj][:])
                # alternate output rings so stores never queue on one FIFO
                eng = nc.sync if ni % 2 == 0 else nc.scalar
                eng.dma_start(out=outW[:, ni, :], in_=st[:])

            for ni in range(NT):
                main_tile(ni)

    nc.compile()
    return nc


def _get_nc():
    if "nc" not in _CACHE:
        _CACHE["nc"] = _build()
    return _CACHE["nc"]


def _in_maps(z, mu, log_cov_diag, prior_logits):
    z = np.ascontiguousarray(np.asarray(z, dtype=np.float32))
    mu = np.asarray(mu, dtype=np.float32)
    lc = np.asarray(log_cov_diag, dtype=np.float32).reshape(1, D)
    pl = np.asarray(prior_logits, dtype=np.float32).reshape(1, C)
    # mu[c, kt2*256 + i*128 + p] -> m8[p, kt2, i, c]
    m8 = np.ascontiguousarray(
        mu.reshape(C, KT2, 2, P).transpose(3, 1, 2, 0)
    ).astype(ml_dtypes.float8_e4m3)
    maps = []
    for cc in range(NCORES):
        zc = z[cc * NSH:(cc + 1) * NSH]
        # zc[ch*512 + n, kt*128 + p] -> zB[p, ch, kt, n]
        zB = np.ascontiguousarray(
            zc.reshape(NCH, F, KT, P).transpose(3, 0, 2, 1)
        ).astype(ml_dtypes.bfloat16)
        maps.append({"zB": zB, "m8": m8, "lc": lc, "prior": pl})
    return maps


def _run(z, mu, log_cov_diag, prior_logits, trace=False, **kw):
    nc = _get_nc()
    maps = _in_maps(z, mu, log_cov_diag, prior_logits)
    res = run_bass_kernel_spmd(nc, maps, list(range(NCORES)), trace=trace, **kw)
    parts = []
    for cc in range(NCORES):
        o = res.results[cc]["outW"]          # [P, NT, C] fp16
        parts.append(np.asarray(o).transpose(1, 0, 2).reshape(NSH, C))
    full = np.concatenate(parts, axis=0).astype(np.float32)
    return full, res


def kernel(z, mu, log_cov_diag, prior_logits):
    full, _ = _run(z, mu, log_cov_diag, prior_logits)
    return full


# revision 14
# speedup vs baseline: 1.0921x; 1.0921x over previous
"""Trainium2 Bass kernel for DiagTrainableLDAHead (retrieval_knn).

out[n,c] = log_prior[c] - 0.5*(m2[n,c] + log_det)
m2[n,c]  = sum_d (z[n,d]-mu[c,d])^2 * inv_var[d]
=> out[n,c] = cross[n,c] + rb[n] + cb[c]
   cross = (z*inv_var) @ mu^T      (fp8e4 DoubleRow GEMM, ~155 TF/s)
   rb[n] = -0.5 * sum_d z^2 inv_var          (fp16 PE reduce)
   cb[c] = log_prior[c] - 0.5*(mu_sq[c]+log_det)  (fp8 DR PE reduce)

rb/cb are folded INTO the GEMM as a third DoubleRow matmul on augmented
fp8 operands (rb scaled by 1/8, split hi/mid/lo across k-slots whose
partner holds 8; cb split hi/lo), so PSUM holds the finished output and
eviction is a bare fp32->fp16 copy split across scalar+vector.

Structural points (from trace analysis of earlier revisions):
 - every engine queue is strict FIFO, so instructions are EMITTED in
   expected-readiness order; bulk prep first, slow chains interleaved,
   evicts last.  PE order: warmups -> ch0 z_sq -> mu_sq -> ch0 main
   tiles -> ch1 z_sq -> ch1 main tiles.
 - input DMA rides the sync HWDGE ring (no act-table preamble: first
   bytes ~4.4us, vs ~8.7us on the scalar ring).
 - zaug/baug zero/one/eight templates are DMA'd constants on the
   gpsimd SWDGE ring (fp8 memsets cost ~3.5us each on DVE).
 - log_prior / log_det reductions are partition-parallel ([128,16] /
   [128,4] layouts + PE ones-matmul for cross-partition sums);
   log-softmax is shift-free (safe for |prior| < ~80).
 - outputs alternate scalar/sync rings; 8 PE warmup matmuls hold the
   HAM clock gate open until real work arrives.

Sharding: data-parallel over N across 8 cores; mu/log_cov/prior
replicated; forward-only, no collectives.  Host prep is layout/dtype
only; all input-dependent arithmetic is on-device.
"""
import sys

sys.path.insert(0, "/opt/trn_rl_repo")

import ml_dtypes
import numpy as np

import concourse.bacc as bacc
import concourse.tile as tile
from concourse import mybir
from concourse.bass_utils import run_bass_kernel_spmd

F32 = mybir.dt.float32
F16 = mybir.dt.float16
BF16 = mybir.dt.bfloat16
F8 = mybir.dt.float8e4
AF = mybir.ActivationFunctionType
ALU = mybir.AluOpType
DR = mybir.MatmulPerfMode.DoubleRow

N, C, D = 8192, 2048, 512
NCORES = 8
NSH = N // NCORES          # 1024 rows per core
P = 128
NCH = 2                    # n-chunks of 512
KT = 4                     # 128-wide k-tiles
KT2 = 2                    # 256-wide DoubleRow k-tiles (pairs i=0/1)
CJ = 4                     # c-chunks of 512
F = 512
NT = NSH // P              # 8 output row-tiles
PJ = C // P                # 16 prior columns in [128,16] layout

_CACHE = {}


def _build():
    nc = bacc.Bacc("TRN2", target_bir_lowering=False, debug=False,
                   enable_asserts=False, num_devices=NCORES)

    zB = nc.dram_tensor("zB", [P, NCH, KT, F], BF16, kind="ExternalInput").ap()
    m8 = nc.dram_tensor("m8", [P, KT2, 2, C], F8, kind="ExternalInput").ap()
    zaugT = nc.dram_tensor("zaugT", [P, 2, NSH], F8, kind="ExternalInput").ap()
    baugT = nc.dram_tensor("baugT", [P, 2, C], F8, kind="ExternalInput").ap()
    lc = nc.dram_tensor("lc", [1, D], F32, kind="ExternalInput").ap()
    prior = nc.dram_tensor("prior", [1, C], F32, kind="ExternalInput").ap()
    pr2d = nc.dram_tensor("pr2d", [P, PJ], F32, kind="ExternalInput").ap()
    outW = nc.dram_tensor("outW", [P, NT, C], F16, kind="ExternalOutput").ap()

    with tile.TileContext(nc) as tc:
        with (
            tc.tile_pool(name="const", bufs=1) as const,
            tc.tile_pool(name="stage", bufs=3) as stage,
            tc.tile_pool(name="psS", bufs=2, space="PSUM") as psS,
            tc.tile_pool(name="psM", bufs=6, space="PSUM") as psM,
        ):
            # ---- input DMAs: sync ring (big/critical), gpsimd (templates)
            lc_f = const.tile([1, D], F32)
            pr2 = const.tile([P, PJ], F32)
            pr = const.tile([1, C], F32)
            zF = const.tile([P, NCH, KT, F], BF16)
            m8s = const.tile([P, KT2, 2, C], F8)
            zaug = const.tile([P, 2, NSH], F8)
            baug = const.tile([P, 2, C], F8)
            nc.sync.dma_start(out=lc_f[:], in_=lc[:, :])
            nc.sync.dma_start(out=pr2[:], in_=pr2d[:, :])
            nc.sync.dma_start(out=pr[:], in_=prior[:, :])
            nc.sync.dma_start(out=zF[:, 0], in_=zB[:, 0])
            nc.sync.dma_start(out=m8s[:, 0], in_=m8[:, 0])
            nc.sync.dma_start(out=m8s[:, 1], in_=m8[:, 1])
            nc.sync.dma_start(out=zF[:, 1], in_=zB[:, 1])
            nc.gpsimd.dma_start(out=zaug[:], in_=zaugT[:, :])
            nc.gpsimd.dma_start(out=baug[:], in_=baugT[:, :])

            # ---- tiny consts on DVE (fast, no deps) -------------------
            wz = const.tile([P, F], BF16)
            nc.vector.memset(wz[:], 0.0)
            id1 = const.tile([1, 1], F32)
            nc.vector.memset(id1[:], 1.0)
            ones32 = const.tile([P, 1], F32)
            nc.vector.memset(ones32[:], 1.0)

            # ---- PE: warmup (holds HAM open), lc transposes -----------
            for _ in range(8):
                pw = psS.tile([8, F], F32, tag="setup", name="pw")
                nc.tensor.matmul(pw[:], lhsT=wz[:, 0:8], rhs=wz[:],
                                 start=True, stop=True)
            plc = psS.tile([P, KT], F32, tag="setup")
            for kt in range(KT):
                nc.tensor.transpose(plc[:, kt:kt + 1],
                                    lc_f[:, kt * P:(kt + 1) * P], id1[:])
            for _ in range(2):
                pw = psS.tile([8, F], F32, tag="setup", name="pw")
                nc.tensor.matmul(pw[:], lhsT=wz[:, 0:8], rhs=wz[:],
                                 start=True, stop=True)

            # ---- scalar: iv chain (early; triggers act-table preamble)
            lc_p = const.tile([P, KT], F32)
            nc.scalar.copy(lc_p[:], plc[:])
            iv = const.tile([P, KT], F32)       # exp(-lc)
            nc.scalar.activation(iv[:], lc_p[:], AF.Exp, scale=-1.0)
            iv16 = const.tile([P, KT], F16)
            nc.scalar.copy(iv16[:], iv[:])
            iv8 = const.tile([P, KT2, 2, 16], F8)  # 16B pair stride for DR
            for kt in range(KT):
                nc.scalar.copy(iv8[:, kt // 2, kt % 2, 0:1], iv[:, kt:kt + 1])

            # ---- ch0 z prep: zq on DVE; z8 split scalar/DVE -----------
            zq = const.tile([P, NCH, KT, F], F16)
            z8 = const.tile([P, NCH, KT2, 2, F], F8)

            def prep_z(ch):
                nc.vector.tensor_tensor(zq[:, ch], zF[:, ch], zF[:, ch],
                                        ALU.mult)
                for kt in range(KT):
                    dst = z8[:, ch, kt // 2, kt % 2, :]
                    if kt < 2:
                        nc.scalar.activation(dst, zF[:, ch, kt, :],
                                             AF.Identity,
                                             scale=iv[:, kt:kt + 1])
                    else:
                        nc.vector.tensor_scalar_mul(dst, zF[:, ch, kt, :],
                                                    iv[:, kt:kt + 1])

            prep_z(0)

            # mu squares: k2=0 on scalar, k2=1 on DVE (per cj chunk for
            # fine-grained readiness)
            mq = const.tile([P, KT2, 2, C], F8)
            for cj in range(CJ):
                s = slice(cj * F, (cj + 1) * F)
                nc.scalar.activation(mq[:, 0, :, s], m8s[:, 0, :, s], AF.Square)
                nc.vector.tensor_tensor(mq[:, 1, :, s], m8s[:, 1, :, s],
                                        m8s[:, 1, :, s], ALU.mult)

            # ---- PE: ch0 z_sq, log_det, mu_sq, sum_exp reduces --------
            pz0 = psS.tile([1, F], F32, tag="setup", name="pz0")
            for kt in range(KT):
                nc.tensor.matmul(pz0[:], lhsT=iv16[:, kt:kt + 1],
                                 rhs=zq[:, 0, kt, :],
                                 start=(kt == 0), stop=(kt == KT - 1))
            ldp = const.tile([P, 1], F32)
            nc.vector.tensor_reduce(out=ldp[:], in_=lc_p[:],
                                    axis=mybir.AxisListType.X, op=ALU.add)
            # pld/pse live in the main-pool ring: the setup ring's WAR
            # chain (reduce psum -> cb reader -> nbb -> lse -> pse) would
            # otherwise deadlock
            pld = psM.tile([1, 1], F32, tag="mm")
            nc.tensor.matmul(pld[:], lhsT=ldp[:], rhs=ones32[:],
                             start=True, stop=True)
            pms = []
            for cj in range(CJ):
                s = slice(cj * F, (cj + 1) * F)
                pm = psS.tile([1, F], F32, tag="setup", name="pm")
                for k2 in range(KT2):
                    nc.tensor.matmul(pm[:], lhsT=iv8[:, k2, :, 0:1],
                                     rhs=mq[:, k2, :, s],
                                     start=(k2 == 0), stop=(k2 == KT2 - 1),
                                     perf_mode=DR)
                pms.append(pm)
            # shift-free sum(exp(prior)) in partition-parallel layout
            pex = const.tile([P, PJ], F32)
            nc.scalar.activation(pex[:], pr2[:], AF.Exp)
            sexp = const.tile([P, 1], F32)
            nc.vector.tensor_reduce(out=sexp[:], in_=pex[:],
                                    axis=mybir.AxisListType.X, op=ALU.add)
            pse = psM.tile([1, 1], F32, tag="mm")
            nc.tensor.matmul(pse[:], lhsT=sexp[:], rhs=ones32[:],
                             start=True, stop=True)

            # ---- scalar: nbb = -0.5*log_det - log(sum_exp) ------------
            lse = const.tile([1, 1], F32)
            nc.scalar.activation(lse[:], pse[:], AF.Ln)
            nldh = const.tile([1, 1], F32)
            nc.scalar.mul(nldh[:], pld[:], -0.5)
            nbb = const.tile([1, 1], F32)
            nc.scalar.activation(nbb[:], lse[:], AF.Identity, bias=nldh[:],
                                 scale=-1.0)

            # ---- rb chain ch0 (fills zaug rows for n < 512) -----------
            rb32 = const.tile([1, NSH], F32)
            h32 = const.tile([1, NSH], F32)
            m32 = const.tile([1, NSH], F32)
            m32b = const.tile([1, NSH], F32)
            l32 = const.tile([1, NSH], F32)

            def rb_chain(ch, pz):
                s = slice(ch * F, (ch + 1) * F)
                nc.scalar.activation(rb32[:, s], pz[:], AF.Copy, scale=-0.0625)
                nc.scalar.copy(zaug[0:1, 0, s], rb32[:, s])
                nc.scalar.copy(h32[:, s], zaug[0:1, 0, s])
                nc.vector.tensor_tensor(m32[:, s], rb32[:, s], h32[:, s],
                                        ALU.subtract)
                nc.scalar.copy(zaug[0:1, 1, s], m32[:, s])
                nc.scalar.copy(m32b[:, s], zaug[0:1, 1, s])
                nc.vector.tensor_tensor(l32[:, s], m32[:, s], m32b[:, s],
                                        ALU.subtract)
                nc.scalar.copy(zaug[32:33, 0, s], l32[:, s])

            rb_chain(0, pz0)

            # ---- cb chain (fills baug rows) ---------------------------
            cb32 = const.tile([1, C], F32)
            ch32 = const.tile([1, C], F32)
            cl32 = const.tile([1, C], F32)
            for cj in range(CJ):
                s = slice(cj * F, (cj + 1) * F)
                nc.scalar.activation(cb32[:, s], pms[cj][:], AF.Identity,
                                     bias=nbb[:], scale=-0.5)
                nc.vector.tensor_tensor(cb32[:, s], cb32[:, s], pr[:, s],
                                        ALU.add)
                nc.scalar.copy(baug[32:33, 1, s], cb32[:, s])
                nc.scalar.copy(ch32[:, s], baug[32:33, 1, s])
                nc.vector.tensor_tensor(cl32[:, s], cb32[:, s], ch32[:, s],
                                        ALU.subtract)
                nc.scalar.copy(baug[64:65, 0, s], cl32[:, s])

            # ---- main tiles -------------------------------------------
            def main_tile(ni):
                ch, t = ni // KT, ni % KT
                ns = slice(t * P, (t + 1) * P)
                ps = [psM.tile([P, F], F32, name=f"ps{cj}", tag="mm")
                      for cj in range(CJ)]
                for k2 in range(KT2):
                    for cj in range(CJ):
                        nc.tensor.matmul(
                            ps[cj][:],
                            lhsT=z8[:, ch, k2, :, ns],
                            rhs=m8s[:, k2, :, cj * F:(cj + 1) * F],
                            start=(k2 == 0), stop=False, perf_mode=DR)
                for cj in range(CJ):
                    nc.tensor.matmul(
                        ps[cj][:],
                        lhsT=zaug[:, :, ni * P:(ni + 1) * P],
                        rhs=baug[:, :, cj * F:(cj + 1) * F],
                        start=False, stop=True, perf_mode=DR)
                st = stage.tile([P, C], F16, tag="st", name="st")
                for cj in range(CJ):
                    if cj < 2:
                        nc.scalar.copy(st[:, cj * F:(cj + 1) * F], ps[cj][:])
                    else:
                        nc.vector.tensor_copy(out=st[:, cj * F:(cj + 1) * F],
                                              in_=ps[cj][:])
                eng = nc.scalar if ni % 2 == 0 else nc.sync
                eng.dma_start(out=outW[:, ni, :], in_=st[:])

            # ch0 tiles while ch1 z-prep happens behind them
            for ni in range(0, KT):
                main_tile(ni)

            # ---- ch1 z prep + z_sq + rb, then ch1 tiles ---------------
            prep_z(1)
            pz1 = psS.tile([1, F], F32, tag="setup", name="pz1")
            for kt in range(KT):
                nc.tensor.matmul(pz1[:], lhsT=iv16[:, kt:kt + 1],
                                 rhs=zq[:, 1, kt, :],
                                 start=(kt == 0), stop=(kt == KT - 1))
            rb_chain(1, pz1)
            for ni in range(KT, NT):
                main_tile(ni)

    nc.compile()
    return nc


def _get_nc():
    if "nc" not in _CACHE:
        _CACHE["nc"] = _build()
    return _CACHE["nc"]


def _aug_templates():
    """Constant fp8 templates: zeros + the 1/8 slot constants.
    slot (Ki,i): (0,0) rb_hi*8  (0,1) rb_mid*8  (32,0) rb_lo*8
                 (32,1) 1*cb_hi  (64,0) 1*cb_lo"""
    e = ml_dtypes.float8_e4m3
    za = np.zeros((P, 2, NSH), dtype=e)
    za[32, 1, :] = e(1.0)
    za[64, 0, :] = e(1.0)
    ba = np.zeros((P, 2, C), dtype=e)
    ba[0, :, :] = e(8.0)
    ba[32, 0, :] = e(8.0)
    return za, ba


def _in_maps(z, mu, log_cov_diag, prior_logits):
    z = np.ascontiguousarray(np.asarray(z, dtype=np.float32))
    mu = np.asarray(mu, dtype=np.float32)
    lcv = np.asarray(log_cov_diag, dtype=np.float32).reshape(1, D)
    pl = np.asarray(prior_logits, dtype=np.float32).reshape(1, C)
    pr2 = np.ascontiguousarray(pl.reshape(PJ, P).T)   # pr2[p,j]=prior[j*128+p]
    m8c = np.ascontiguousarray(
        mu.reshape(C, KT2, 2, P).transpose(3, 1, 2, 0)
    ).astype(ml_dtypes.float8_e4m3)
    za, ba = _aug_templates()
    maps = []
    for cc in range(NCORES):
        zc = z[cc * NSH:(cc + 1) * NSH]
        zBc = np.ascontiguousarray(
            zc.reshape(NCH, F, KT, P).transpose(3, 0, 2, 1)
        ).astype(ml_dtypes.bfloat16)
        maps.append({"zB": zBc, "m8": m8c, "zaugT": za, "baugT": ba,
                     "lc": lcv, "prior": pl, "pr2d": pr2})
    return maps


def _run(z, mu, log_cov_diag, prior_logits, trace=False, **kw):
    nc = _get_nc()
    maps = _in_maps(z, mu, log_cov_diag, prior_logits)
    res = run_bass_kernel_spmd(nc, maps, list(range(NCORES)), trace=trace, **kw)
    parts = []
    for cc in range(NCORES):
        o = res.results[cc]["outW"]          # [P, NT, C] fp16
        parts.append(np.asarray(o).transpose(1, 0, 2).reshape(NSH, C))
    full = np.concatenate(parts, axis=0).astype(np.float32)
    return full, res


def kernel(z, mu, log_cov_diag, prior_logits):
    full, _ = _run(z, mu, log_cov_diag, prior_logits)
    return full


# revision 15
# speedup vs baseline: 1.1459x; 1.0492x over previous
"""Trainium2 Bass kernel for DiagTrainableLDAHead (retrieval_knn).

out[n,c] = log_prior[c] - 0.5*(m2[n,c] + log_det)
m2[n,c]  = sum_d (z[n,d]-mu[c,d])^2 * inv_var[d]
=> out[n,c] = cross[n,c] + rb[n] + cb[c]
   cross = (z*inv_var) @ mu^T      (fp8e4 DoubleRow GEMM, ~155 TF/s)
   rb[n] = -0.5 * sum_d z^2 inv_var          (fp16 PE reduce)
   cb[c] = log_prior[c] - 0.5*(mu_sq[c]+log_det)  (fp8 DR PE reduce)

rb/cb are folded INTO the GEMM as a third DoubleRow matmul on augmented
fp8 operands (rb scaled by 1/8, split hi/mid/lo across k-slots whose
partner holds 8; cb split hi/lo), so PSUM holds the finished output and
eviction is a bare fp32->fp16 copy split across scalar+vector.

Hard-won trace facts this revision encodes:
 - first DMA byte lands ~8.7us regardless of ring (fixed runtime tax),
   so ~20 PE warmup matmuls bridge t=0..9us and keep the HAM clock
   gate at 2.4GHz (cold matmuls run 1.75x slower);
 - inputs split across the sync AND scalar HWDGE rings in parallel;
   outputs all ride sync (a dma_start's semaphore wait head-of-line
   blocks its whole engine queue, so outs never sit on compute queues);
 - every engine queue is strict FIFO: all emissions are ordered by
   expected readiness (z-prep ch0 -> mu squares -> z-prep ch1 ->
   scalar chains -> evicts);
 - zaug/baug templates are DMA'd constants on the gpsimd SWDGE ring;
 - log_det / sum(exp(prior)) are partition-parallel + PE ones-matmul;
   their [1,1] psums go EARLY in the setup ring to keep WAR chains
   acyclic (a late slot once deadlocked against the cb chain).

Sharding: data-parallel over N across 8 cores; mu/log_cov/prior
replicated; forward-only, no collectives.  Host prep is layout/dtype
only; all input-dependent arithmetic is on-device.
"""
import sys

sys.path.insert(0, "/opt/trn_rl_repo")

import ml_dtypes
import numpy as np

import concourse.bacc as bacc
import concourse.tile as tile
from concourse import mybir
from concourse.bass_utils import run_bass_kernel_spmd

F32 = mybir.dt.float32
F16 = mybir.dt.float16
BF16 = mybir.dt.bfloat16
F8 = mybir.dt.float8e4
AF = mybir.ActivationFunctionType
ALU = mybir.AluOpType
DR = mybir.MatmulPerfMode.DoubleRow

N, C, D = 8192, 2048, 512
NCORES = 8
NSH = N // NCORES          # 1024 rows per core
P = 128
NCH = 2                    # n-chunks of 512
KT = 4                     # 128-wide k-tiles
KT2 = 2                    # 256-wide DoubleRow k-tiles (pairs i=0/1)
CJ = 4                     # c-chunks of 512
F = 512
NT = NSH // P              # 8 output row-tiles
PJ = C // P                # 16 prior columns in [128,16] layout

_CACHE = {}


def _build():
    nc = bacc.Bacc("TRN2", target_bir_lowering=False, debug=False,
                   enable_asserts=False, num_devices=NCORES)

    zB = nc.dram_tensor("zB", [P, NCH, KT, F], BF16, kind="ExternalInput").ap()
    m8 = nc.dram_tensor("m8", [P, KT2, 2, C], F8, kind="ExternalInput").ap()
    zaugT = nc.dram_tensor("zaugT", [P, 2, NSH], F8, kind="ExternalInput").ap()
    baugT = nc.dram_tensor("baugT", [P, 2, C], F8, kind="ExternalInput").ap()
    lc = nc.dram_tensor("lc", [1, D], F32, kind="ExternalInput").ap()
    prior = nc.dram_tensor("prior", [1, C], F32, kind="ExternalInput").ap()
    pr2d = nc.dram_tensor("pr2d", [P, PJ], F32, kind="ExternalInput").ap()
    outW = nc.dram_tensor("outW", [P, NT, C], F16, kind="ExternalOutput").ap()

    with tile.TileContext(nc) as tc:
        with (
            tc.tile_pool(name="const", bufs=1) as const,
            tc.tile_pool(name="stage", bufs=4) as stage,
            tc.tile_pool(name="psS", bufs=2, space="PSUM") as psS,
            tc.tile_pool(name="psM", bufs=6, space="PSUM") as psM,
        ):
            # ---- input DMAs: sync + scalar rings in parallel ----------
            lc_f = const.tile([1, D], F32)
            pr2 = const.tile([P, PJ], F32)
            pr = const.tile([1, C], F32)
            zF = const.tile([P, NCH, KT, F], BF16)
            m8s = const.tile([P, KT2, 2, C], F8)
            zaug = const.tile([P, 2, NSH], F8)
            baug = const.tile([P, 2, C], F8)
            nc.sync.dma_start(out=lc_f[:], in_=lc[:, :])
            nc.sync.dma_start(out=pr2[:], in_=pr2d[:, :])
            nc.sync.dma_start(out=pr[:], in_=prior[:, :])
            nc.sync.dma_start(out=zF[:, 0], in_=zB[:, 0])
            nc.sync.dma_start(out=m8s[:, 0], in_=m8[:, 0])
            nc.scalar.dma_start(out=m8s[:, 1], in_=m8[:, 1])
            nc.scalar.dma_start(out=zF[:, 1], in_=zB[:, 1])
            nc.gpsimd.dma_start(out=zaug[:], in_=zaugT[:, :])
            nc.gpsimd.dma_start(out=baug[:], in_=baugT[:, :])

            # ---- tiny consts on DVE (fast, no deps) -------------------
            wz = const.tile([P, F], BF16)
            nc.vector.memset(wz[:], 0.0)
            id1 = const.tile([1, 1], F32)
            nc.vector.memset(id1[:], 1.0)
            ones32 = const.tile([P, 1], F32)
            nc.vector.memset(ones32[:], 1.0)

            # ---- PE: warmup bridging the ~9us DMA-init window ---------
            def warm(k):
                for _ in range(k):
                    pw = psS.tile([8, F], F32, tag="setup", name="pw")
                    nc.tensor.matmul(pw[:], lhsT=wz[:, 0:8], rhs=wz[:],
                                     start=True, stop=True)

            warm(20)
            plc = psS.tile([P, KT], F32, tag="setup")
            for kt in range(KT):
                nc.tensor.transpose(plc[:, kt:kt + 1],
                                    lc_f[:, kt * P:(kt + 1) * P], id1[:])
            warm(2)

            # ---- scalar: iv chain -------------------------------------
            lc_p = const.tile([P, KT], F32)
            nc.scalar.copy(lc_p[:], plc[:])
            iv = const.tile([P, KT], F32)       # exp(-lc)
            nc.scalar.activation(iv[:], lc_p[:], AF.Exp, scale=-1.0)
            iv16 = const.tile([P, KT], F16)
            nc.scalar.copy(iv16[:], iv[:])
            iv8 = const.tile([P, KT2, 2, 16], F8)  # 16B pair stride for DR
            for kt in range(KT):
                nc.scalar.copy(iv8[:, kt // 2, kt % 2, 0:1], iv[:, kt:kt + 1])

            # ---- log_det / sum_exp psums: EARLY in the setup ring -----
            ldp = const.tile([P, 1], F32)
            nc.vector.tensor_reduce(out=ldp[:], in_=lc_p[:],
                                    axis=mybir.AxisListType.X, op=ALU.add)
            pld = psS.tile([1, 1], F32, tag="setup")
            nc.tensor.matmul(pld[:], lhsT=ldp[:], rhs=ones32[:],
                             start=True, stop=True)
            pex = const.tile([P, PJ], F32)
            nc.scalar.activation(pex[:], pr2[:], AF.Exp)
            sexp = const.tile([P, 1], F32)
            nc.vector.tensor_reduce(out=sexp[:], in_=pex[:],
                                    axis=mybir.AxisListType.X, op=ALU.add)
            pse = psS.tile([1, 1], F32, tag="setup")
            nc.tensor.matmul(pse[:], lhsT=sexp[:], rhs=ones32[:],
                             start=True, stop=True)

            # ---- z/mu prep, readiness-ordered -------------------------
            zq = const.tile([P, NCH, KT, F], F16)
            z8 = const.tile([P, NCH, KT2, 2, F], F8)
            mq = const.tile([P, KT2, 2, C], F8)

            def z8_scalar(ch):        # kt 0,1 on scalar
                for kt in (0, 1):
                    nc.scalar.activation(z8[:, ch, kt // 2, kt % 2, :],
                                         zF[:, ch, kt, :], AF.Identity,
                                         scale=iv[:, kt:kt + 1])

            def z8_vector(ch):        # kt 2,3 on DVE
                for kt in (2, 3):
                    nc.vector.tensor_scalar_mul(
                        z8[:, ch, kt // 2, kt % 2, :], zF[:, ch, kt, :],
                        iv[:, kt:kt + 1])

            # DVE: zq0 -> z80cd -> mq-k1 per cj -> (subtracts come later)
            nc.vector.tensor_tensor(zq[:, 0], zF[:, 0], zF[:, 0], ALU.mult)
            z8_vector(0)
            # scalar: z80ab -> mq-k0 -> nbb chain
            z8_scalar(0)
            for cj in range(CJ):
                s = slice(cj * F, (cj + 1) * F)
                nc.scalar.activation(mq[:, 0, :, s], m8s[:, 0, :, s], AF.Square)
            for cj in range(CJ):
                s = slice(cj * F, (cj + 1) * F)
                nc.vector.tensor_tensor(mq[:, 1, :, s], m8s[:, 1, :, s],
                                        m8s[:, 1, :, s], ALU.mult)

            lse = const.tile([1, 1], F32)
            nc.scalar.activation(lse[:], pse[:], AF.Ln)
            nldh = const.tile([1, 1], F32)
            nc.scalar.mul(nldh[:], pld[:], -0.5)
            nbb = const.tile([1, 1], F32)
            nc.scalar.activation(nbb[:], lse[:], AF.Identity, bias=nldh[:],
                                 scale=-1.0)

            # ---- PE: z_sq ch0, mu_sq (k0 first, k1 as mq-k1 lands) ----
            pz0 = psS.tile([1, F], F32, tag="setup", name="pz0")
            for kt in range(KT):
                nc.tensor.matmul(pz0[:], lhsT=iv16[:, kt:kt + 1],
                                 rhs=zq[:, 0, kt, :],
                                 start=(kt == 0), stop=(kt == KT - 1))
            pms = [psS.tile([1, F], F32, tag="setup", name=f"pm{cj}")
                   for cj in range(CJ)]
            for cj in range(CJ):
                nc.tensor.matmul(pms[cj][:], lhsT=iv8[:, 0, :, 0:1],
                                 rhs=mq[:, 0, :, cj * F:(cj + 1) * F],
                                 start=True, stop=False, perf_mode=DR)

            # ---- scalar: z81ab early; DVE: zq1/z81cd ------------------
            z8_scalar(1)
            nc.vector.tensor_tensor(zq[:, 1], zF[:, 1], zF[:, 1], ALU.mult)
            z8_vector(1)

            for cj in range(CJ):
                nc.tensor.matmul(pms[cj][:], lhsT=iv8[:, 1, :, 0:1],
                                 rhs=mq[:, 1, :, cj * F:(cj + 1) * F],
                                 start=False, stop=True, perf_mode=DR)

            # ---- rb chain ch0 -----------------------------------------
            rb32 = const.tile([1, NSH], F32)
            h32 = const.tile([1, NSH], F32)
            m32 = const.tile([1, NSH], F32)
            m32b = const.tile([1, NSH], F32)
            l32 = const.tile([1, NSH], F32)

            def rb_chain(ch, pz):
                s = slice(ch * F, (ch + 1) * F)
                nc.scalar.activation(rb32[:, s], pz[:], AF.Copy, scale=-0.0625)
                nc.scalar.copy(zaug[0:1, 0, s], rb32[:, s])
                nc.scalar.copy(h32[:, s], zaug[0:1, 0, s])
                nc.vector.tensor_tensor(m32[:, s], rb32[:, s], h32[:, s],
                                        ALU.subtract)
                nc.scalar.copy(zaug[0:1, 1, s], m32[:, s])
                nc.scalar.copy(m32b[:, s], zaug[0:1, 1, s])
                nc.vector.tensor_tensor(l32[:, s], m32[:, s], m32b[:, s],
                                        ALU.subtract)
                nc.scalar.copy(zaug[32:33, 0, s], l32[:, s])

            rb_chain(0, pz0)

            # ---- cb chain ---------------------------------------------
            cb32 = const.tile([1, C], F32)
            ch32 = const.tile([1, C], F32)
            cl32 = const.tile([1, C], F32)
            for cj in range(CJ):
                s = slice(cj * F, (cj + 1) * F)
                nc.scalar.activation(cb32[:, s], pms[cj][:], AF.Identity,
                                     bias=nbb[:], scale=-0.5)
                nc.vector.tensor_tensor(cb32[:, s], cb32[:, s], pr[:, s],
                                        ALU.add)
                nc.scalar.copy(baug[32:33, 1, s], cb32[:, s])
                nc.scalar.copy(ch32[:, s], baug[32:33, 1, s])
                nc.vector.tensor_tensor(cl32[:, s], cb32[:, s], ch32[:, s],
                                        ALU.subtract)
                nc.scalar.copy(baug[64:65, 0, s], cl32[:, s])

            # ---- z_sq ch1 + rb ch1 ------------------------------------
            pz1 = psS.tile([1, F], F32, tag="setup", name="pz1")
            for kt in range(KT):
                nc.tensor.matmul(pz1[:], lhsT=iv16[:, kt:kt + 1],
                                 rhs=zq[:, 1, kt, :],
                                 start=(kt == 0), stop=(kt == KT - 1))
            rb_chain(1, pz1)

            # ---- main tiles -------------------------------------------
            def main_tile(ni):
                ch, t = ni // KT, ni % KT
                ns = slice(t * P, (t + 1) * P)
                ps = [psM.tile([P, F], F32, name=f"ps{cj}", tag="mm")
                      for cj in range(CJ)]
                for k2 in range(KT2):
                    for cj in range(CJ):
                        nc.tensor.matmul(
                            ps[cj][:],
                            lhsT=z8[:, ch, k2, :, ns],
                            rhs=m8s[:, k2, :, cj * F:(cj + 1) * F],
                            start=(k2 == 0), stop=False, perf_mode=DR)
                for cj in range(CJ):
                    nc.tensor.matmul(
                        ps[cj][:],
                        lhsT=zaug[:, :, ni * P:(ni + 1) * P],
                        rhs=baug[:, :, cj * F:(cj + 1) * F],
                        start=False, stop=True, perf_mode=DR)
                st = stage.tile([P, C], F16, tag="st", name="st")
                for cj in range(CJ):
                    if cj < 2:
                        nc.scalar.copy(st[:, cj * F:(cj + 1) * F], ps[cj][:])
                    else:
                        nc.vector.tensor_copy(out=st[:, cj * F:(cj + 1) * F],
                                              in_=ps[cj][:])
                nc.sync.dma_start(out=outW[:, ni, :], in_=st[:])

            for ni in range(NT):
                main_tile(ni)

    nc.compile()
    return nc


def _get_nc():
    if "nc" not in _CACHE:
        _CACHE["nc"] = _build()
    return _CACHE["nc"]


def _aug_templates():
    """Constant fp8 templates: zeros + the 1/8 slot constants.
    slot (Ki,i): (0,0) rb_hi*8  (0,1) rb_mid*8  (32,0) rb_lo*8
                 (32,1) 1*cb_hi  (64,0) 1*cb_lo"""
    e = ml_dtypes.float8_e4m3
    za = np.zeros((P, 2, NSH), dtype=e)
    za[32, 1, :] = e(1.0)
    za[64, 0, :] = e(1.0)
    ba = np.zeros((P, 2, C), dtype=e)
    ba[0, :, :] = e(8.0)
    ba[32, 0, :] = e(8.0)
    return za, ba


def _in_maps(z, mu, log_cov_diag, prior_logits):
    z = np.ascontiguousarray(np.asarray(z, dtype=np.float32))
    mu = np.asarray(mu, dtype=np.float32)
    lcv = np.asarray(log_cov_diag, dtype=np.float32).reshape(1, D)
    pl = np.asarray(prior_logits, dtype=np.float32).reshape(1, C)
    pr2 = np.ascontiguousarray(pl.reshape(PJ, P).T)   # pr2[p,j]=prior[j*128+p]
    m8c = np.ascontiguousarray(
        mu.reshape(C, KT2, 2, P).transpose(3, 1, 2, 0)
    ).astype(ml_dtypes.float8_e4m3)
    za, ba = _aug_templates()
    maps = []
    for cc in range(NCORES):
        zc = z[cc * NSH:(cc + 1) * NSH]
        zBc = np.ascontiguousarray(
            zc.reshape(NCH, F, KT, P).transpose(3, 0, 2, 1)
        ).astype(ml_dtypes.bfloat16)
        maps.append({"zB": zBc, "m8": m8c, "zaugT": za, "baugT": ba,
                     "lc": lcv, "prior": pl, "pr2d": pr2})
    return maps


def _run(z, mu, log_cov_diag, prior_logits, trace=False, **kw):
    nc = _get_nc()
    maps = _in_maps(z, mu, log_cov_diag, prior_logits)
    res = run_bass_kernel_spmd(nc, maps, list(range(NCORES)), trace=trace, **kw)
    parts = []
    for cc in range(NCORES):
        o = res.results[cc]["outW"]          # [P, NT, C] fp16
        parts.append(np.asarray(o).transpose(1, 0, 2).reshape(NSH, C))
    full = np.concatenate(parts, axis=0).astype(np.float32)
    return full, res


def kernel(z, mu, log_cov_diag, prior_logits):
    full, _ = _run(z, mu, log_cov_diag, prior_logits)
    return full
